# revision 6
# baseline (speedup 1.0000x reference)
"""Trainium2 Bass kernel for the nn_Decoder LSTM-decoder problem.

Reference computation (per agent, 12 steps):
    gates = dec_in @ w_ih.T + h @ w_hh.T + (b_ih + b_hh)
    i, f, g, o = split(gates); c = sig(f)*c + sig(i)*tanh(g); h = sig(o)*tanh(c)
    rel = h @ w_hp.T + b_hp; dec_in = rel @ w_se.T + b_se
Output: rel per step, [12, N, 2].

Key algebraic fusion: dec_in_t is a linear function of h_t, so for steps >= 2
    gates_t = h_{t-1} @ W_eff.T + b_eff,  W_eff = w_hh + w_ih @ w_se @ w_hp
and step 1 uses w_hh plus U = w_ih @ w_se applied to last_pos_rel.
last_pos is dead (never affects the output).

Distribution: pure data parallel over the agent axis, 8192 agents per core
on 8 NeuronCores; weights replicated.

On-chip layout: [feature partitions, agent free], 512-agent chunks.
PE does float32r matmuls (full rate at N=512); ACT does sigmoid/tanh
(single table set); DVE+GPSIMD split the cell-update elementwise work.
"""

import sys

if "/opt/trn_rl_repo" not in sys.path:
    sys.path.insert(0, "/opt/trn_rl_repo")

import numpy as np

T = 12          # steps
H = 128         # hidden dim
NCORES = 8
NPC = 8192      # agents per core
CH = 512        # agents per chunk
NCHUNK = NPC // CH

_CACHE = {}


def _build_program(npc):
    import concourse.bass as bass
    import concourse.tile as tile
    from concourse import bacc, mybir

    dt = mybir.dt
    f32 = dt.float32
    f32r = dt.float32r
    Act = mybir.ActivationFunctionType

    nchunk = npc // CH

    nc = bacc.Bacc(
        "TRN2",
        target_bir_lowering=False,
        debug=False,
        num_devices=NCORES,
    )

    def din(name, shape, dt_=None):
        return nc.dram_tensor(
            name, list(shape), dt_ or f32, kind="ExternalInput"
        ).ap()

    h0_d = din("h0", [npc, H])
    c0_d = din("c0", [npc, H])
    lpr_d = din("lpr", [npc, 2], f32r)
    # lhsT layouts, K on partitions. Gate-bank order is [i, f, o, g].
    wg_d = din("wg", [H, 4 * H], f32r)       # W_eff.T, bank-ordered columns
    whh_d = din("whh", [H, 4 * H], f32r)     # w_hh.T, bank-ordered (step 1)
    u_d = din("u", [2, 4 * H], f32r)         # U.T = (w_ih @ w_se).T, bank-ordered
    bifo_d = din("bifo", [1, 3 * H], f32r)   # b_eff for banks i,f,o (steps 2+)
    b1ifo_d = din("b1ifo", [1, 3 * H], f32r)  # step-1 bias for banks i,f,o
    bg_d = din("bg", [H, 1])           # b_eff g-bank, ACT bias (steps 2+)
    b1g_d = din("b1g", [H, 1])         # step-1 g-bank bias
    whp_d = din("whp", [H, 2], f32r)         # w_hp.T
    bhp_d = din("bhp", [2, 1])
    ident_d = din("ident", [H, H])
    ones_d = din("ones", [1, CH], f32r)
    out_d = nc.dram_tensor("out", [T, npc, 2], f32, kind="ExternalOutput").ap()

    with tile.TileContext(nc) as tc:
        with (
            tc.tile_pool(name="wpool", bufs=1) as wp,
            tc.tile_pool(name="state", bufs=1) as state,
            tc.tile_pool(name="stage", bufs=8) as stage,
            tc.tile_pool(name="sig", bufs=3) as sigp,
            tc.tile_pool(name="tmp", bufs=3) as tmpp,
            tc.tile_pool(name="ps", bufs=2, space="PSUM") as psp,
        ):
            def wtile(ap, shape, tag, dt_=None):
                t_ = wp.tile(list(shape), dt_ or f32, tag=tag)
                nc.sync.dma_start(t_[:], ap)
                return t_

            wg = wtile(wg_d, [H, 4 * H], "wg", f32r)
            whh = wtile(whh_d, [H, 4 * H], "whh", f32r)
            u = wtile(u_d, [2, 4 * H], "u", f32r)
            bifo = wtile(bifo_d, [1, 3 * H], "bifo", f32r)
            b1ifo = wtile(b1ifo_d, [1, 3 * H], "b1ifo", f32r)
            bg = wtile(bg_d, [H, 1], "bg")
            b1g = wtile(b1g_d, [H, 1], "b1g")
            whp = wtile(whp_d, [H, 2], "whp", f32r)
            bhp = wtile(bhp_d, [2, 1], "bhp")
            ident = wtile(ident_d, [H, H], "ident")
            ones = wtile(ones_d, [1, CH], "ones", f32r)
            # last_pos_rel transposed [2, npc] via strided (rearranged) DMA
            lprT = wp.tile([2, npc], f32r, tag="lprT")
            nc.sync.dma_start(lprT[:], lpr_d.rearrange("n k -> k n"))

            h_sb = state.tile([H, npc], f32r, tag="h")
            c_sb = state.tile([H, npc], f32, tag="c")

            # ---- prologue: transpose h0, c0 into [feature, agent] layout ----
            for cki in range(nchunk):
                pt = psp.tile([128, 2048], f32, tag="ps")
                for j in range(4):
                    st = stage.tile([128, H], f32, tag="st_h")
                    rows = slice(cki * CH + j * 128, cki * CH + (j + 1) * 128)
                    nc.sync.dma_start(st[:], h0_d[rows, :])
                    nc.tensor.transpose(
                        pt[:, j * 128:(j + 1) * 128], st[:], ident[:]
                    )
                for j in range(4):
                    st = stage.tile([128, H], f32, tag="st_c")
                    rows = slice(cki * CH + j * 128, cki * CH + (j + 1) * 128)
                    nc.sync.dma_start(st[:], c0_d[rows, :])
                    nc.tensor.transpose(
                        pt[:, 512 + j * 128: 512 + (j + 1) * 128], st[:], ident[:]
                    )
                cols = slice(cki * CH, (cki + 1) * CH)
                nc.vector.tensor_copy(h_sb[:, cols], pt[:, 0:512])
                nc.vector.tensor_copy(c_sb[:, cols], pt[:, 512:1024])

            # ---- main recurrence ----
            for t in range(T):
                first = t == 0
                W = whh if first else wg
                bias_ifo = b1ifo if first else bifo
                bias_g = b1g if first else bg
                for cki in range(nchunk):
                    cols = slice(cki * CH, (cki + 1) * CH)
                    h_ck = h_sb[:, cols]
                    c_ck = c_sb[:, cols]
                    gp = psp.tile([128, 2048], f32, tag="ps")
                    rhs = h_ck
                    for b in range(4):
                        bank = slice(b * CH, (b + 1) * CH)
                        wsl = slice(b * H, (b + 1) * H)
                        n_mm = (2 if b < 3 else 1) + (1 if first else 0)
                        k = 0
                        if b < 3:
                            nc.tensor.matmul(
                                gp[:, bank],
                                bias_ifo[:, wsl],
                                ones[:],
                                start=True, stop=(k == n_mm - 1),
                            )
                            k += 1
                        if first:
                            nc.tensor.matmul(
                                gp[:, bank],
                                u[:, wsl],
                                lprT[:, cols],
                                start=(k == 0), stop=(k == n_mm - 1),
                            )
                            k += 1
                        nc.tensor.matmul(
                            gp[:, bank],
                            W[:, wsl],
                            rhs,
                            start=(k == 0), stop=True,
                        )

                    # activations: banks [i, f, o] -> sigmoid, bank g -> tanh
                    sifo = sigp.tile([128, 3 * CH], f32, tag="sifo")
                    nc.scalar.activation(sifo[:], gp[:, 0:3 * CH], Act.Sigmoid)
                    tg = sigp.tile([128, CH], f32, tag="tg")
                    nc.scalar.activation(
                        tg[:], gp[:, 3 * CH:4 * CH], Act.Tanh, bias=bias_g[:, 0:1]
                    )

                    # cell update
                    m1 = tmpp.tile([128, CH], f32, tag="m1")
                    nc.vector.tensor_mul(m1[:], sifo[:, CH:2 * CH], c_ck)
                    m2 = tmpp.tile([128, CH], f32, tag="m2")
                    nc.vector.tensor_mul(m2[:], sifo[:, 0:CH], tg[:])
                    nc.gpsimd.tensor_add(c_ck, m1[:], m2[:])
                    tcell = sigp.tile([128, CH], f32, tag="tc")
                    nc.scalar.activation(tcell[:], c_ck, Act.Tanh)
                    nc.gpsimd.tensor_mul(h_ck, sifo[:, 2 * CH:3 * CH], tcell[:])

                    # rel = w_hp @ h + b_hp -> rels rows [2t, 2t+2)
                    nc.tensor.matmul(
                        gp[0:2, 0:CH],
                        whp[:],
                        h_ck,
                        start=True, stop=True,
                    )
                    rel_t = tmpp.tile([2, CH], f32, tag="rel")
                    nc.vector.tensor_scalar_add(
                        rel_t[:], gp[0:2, 0:CH], bhp[:, 0:1]
                    )
                    nc.sync.dma_start(
                        out_d[t][cols, :].rearrange("n k -> k n"), rel_t[:]
                    )

    nc.compile()
    return nc


def _fold_weights(w_ih, w_hh, b_ih, b_hh, w_se, b_se, w_hp, b_hp):
    """Host-side constant folding. Gate-bank order [i, f, o, g] (torch order
    in the 4H rows is i, f, g, o)."""
    perm = np.concatenate([
        np.arange(0, H), np.arange(H, 2 * H),
        np.arange(3 * H, 4 * H), np.arange(2 * H, 3 * H),
    ])
    W_eff = w_hh + w_ih @ w_se @ w_hp                      # [4H, H]
    b_eff = (b_hp @ w_se.T + b_se) @ w_ih.T + b_ih + b_hh  # [4H]
    U = w_ih @ w_se                                        # [4H, 2]
    b1 = b_se @ w_ih.T + b_ih + b_hh                       # [4H]

    Wp, bp = W_eff[perm], b_eff[perm]
    Whhp, Up, b1p = w_hh[perm], U[perm], b1[perm]
    f = np.float32
    return {
        "wg": np.ascontiguousarray(Wp.T, f),
        "whh": np.ascontiguousarray(Whhp.T, f),
        "u": np.ascontiguousarray(Up.T, f),
        "bifo": np.ascontiguousarray(bp[:3 * H][None, :], f),
        "b1ifo": np.ascontiguousarray(b1p[:3 * H][None, :], f),
        "bg": np.ascontiguousarray(bp[3 * H:][:, None], f),
        "b1g": np.ascontiguousarray(b1p[3 * H:][:, None], f),
        "whp": np.ascontiguousarray(w_hp.T, f),
        "bhp": np.ascontiguousarray(np.asarray(b_hp)[:, None], f),
        "ident": np.eye(H, dtype=f),
        "ones": np.ones((1, CH), f),
    }


def kernel(last_pos, last_pos_rel, h0, c0,
           w_ih, w_hh, b_ih, b_hh, w_se, b_se, w_hp, b_hp):
    last_pos_rel = np.ascontiguousarray(np.asarray(last_pos_rel), np.float32)
    h0 = np.ascontiguousarray(np.asarray(h0), np.float32)
    c0 = np.ascontiguousarray(np.asarray(c0), np.float32)
    consts = _fold_weights(
        np.asarray(w_ih, np.float32), np.asarray(w_hh, np.float32),
        np.asarray(b_ih, np.float32), np.asarray(b_hh, np.float32),
        np.asarray(w_se, np.float32), np.asarray(b_se, np.float32),
        np.asarray(w_hp, np.float32), np.asarray(b_hp, np.float32),
    )

    npeds = h0.shape[0]
    npc = npeds // NCORES
    if "nc" not in _CACHE or _CACHE.get("npc") != npc:
        _CACHE["nc"] = _build_program(npc)
        _CACHE["npc"] = npc
    nc = _CACHE["nc"]

    in_maps = []
    for ci in range(NCORES):
        rows = slice(ci * npc, (ci + 1) * npc)
        m = {"h0": h0[rows], "c0": c0[rows], "lpr": last_pos_rel[rows]}
        m.update(consts)
        in_maps.append(m)

    from concourse.bass_utils import run_bass_kernel_spmd
    import os

    res = run_bass_kernel_spmd(
        nc, in_maps, list(range(NCORES)),
        tmpdir=os.environ.get("KERNEL_TRACE_DIR"),
    )
    _CACHE["exec_time_ns"] = res.exec_time_ns
    _CACHE["results"] = res
    outs = [np.asarray(res.results[i]["out"]) for i in range(NCORES)]
    return np.concatenate(outs, axis=1)


# revision 9
# speedup vs baseline: 2.4565x; 2.4565x over previous
"""Trainium2 Bass kernel for the nn_Decoder LSTM-decoder problem.

Reference computation (per agent, 12 steps):
    gates = dec_in @ w_ih.T + h @ w_hh.T + (b_ih + b_hh)
    i, f, g, o = split(gates); c = sig(f)*c + sig(i)*tanh(g); h = sig(o)*tanh(c)
    rel = h @ w_hp.T + b_hp; dec_in = rel @ w_se.T + b_se
Output: rel per step, [12, N, 2].

Key algebraic fusion: dec_in_t is a linear function of h_t, so for steps >= 2
    gates_t = h_{t-1} @ W_eff.T + b_eff,  W_eff = w_hh + w_ih @ w_se @ w_hp
and step 1 uses w_hh plus U = w_ih @ w_se applied to last_pos_rel.
last_pos is dead (never affects the output).

Distribution: pure data parallel over the agent axis, 8192 agents per core
on 8 NeuronCores; weights replicated.

On-chip layout: [feature partitions, agent free]. Agents are processed in
pairs of 512-agent chunks (1024 agents per PSUM gate tile) so each ACT
instruction covers 1024 elements per lane with a per-gate per-partition
bias. PE does float32r matmuls; DVE+GPSIMD split the cell-update
elementwise work by columns. The per-step rel output is re-blocked via
SBUF->SBUF DMA and pair-interleaved on DVE so the final DRAM write has
512-byte contiguous runs spread across all 16 DMA ports.
"""

import sys

if "/opt/trn_rl_repo" not in sys.path:
    sys.path.insert(0, "/opt/trn_rl_repo")

import numpy as np

T = 12          # steps
H = 128         # hidden dim
NCORES = 8
NPC = 8192      # agents per core
CH = 512        # agents per chunk (one PSUM bank at fp32)
PAIR = 2 * CH   # agents per gate-tile

_CACHE = {}


def _build_program(npc):
    import concourse.bass as bass
    import concourse.tile as tile
    from concourse import bacc, mybir

    dt = mybir.dt
    f32 = dt.float32
    f32r = dt.float32r
    Act = mybir.ActivationFunctionType

    npair = npc // PAIR
    assert npc % PAIR == 0
    nblk = npc // 64          # agents per partition block in the output stage

    nc = bacc.Bacc(
        "TRN2",
        target_bir_lowering=False,
        debug=False,
        num_devices=NCORES,
    )

    def din(name, shape, dt_=None):
        return nc.dram_tensor(
            name, list(shape), dt_ or f32, kind="ExternalInput"
        ).ap()

    h0_d = din("h0", [npc, H])
    c0_d = din("c0", [npc, H])
    lpr_d = din("lpr", [npc, 2])
    # lhsT layouts, K on partitions. Gate order [i, f, o, g].
    wg_d = din("wg", [H, 4 * H], f32r)    # W_eff.T columns gate-ordered
    whh_d = din("whh", [H, 4 * H], f32r)  # w_hh.T (step 1)
    u_d = din("u", [2, 4 * H], f32r)      # (w_ih @ w_se).T (step 1)
    bias_d = din("bias", [H, 8])          # ACT bias: [b_eff | b1] x [i,f,o,g]
    whp_d = din("whp", [H, 2], f32r)      # w_hp.T
    bhp_d = din("bhp", [2, 1])
    ident_d = din("ident", [H, H])
    out_d = nc.dram_tensor("out", [T, npc, 2], f32, kind="ExternalOutput").ap()

    with tile.TileContext(nc) as tc:
        with (
            tc.tile_pool(name="wpool", bufs=1) as wp,
            tc.tile_pool(name="state", bufs=1) as state,
            tc.tile_pool(name="stage", bufs=4) as stage,
            tc.tile_pool(name="sig", bufs=2) as sigp,
            tc.tile_pool(name="tmp", bufs=2) as tmpp,
            tc.tile_pool(name="outp", bufs=2) as outp,
            tc.tile_pool(name="ps", bufs=4, space="PSUM") as psp,
        ):
            def wtile(ap, shape, tag, dt_=None):
                t_ = wp.tile(list(shape), dt_ or f32, tag=tag)
                nc.sync.dma_start(t_[:], ap)
                return t_

            wg = wtile(wg_d, [H, 4 * H], "wg", f32r)
            whh = wtile(whh_d, [H, 4 * H], "whh", f32r)
            u = wtile(u_d, [2, 4 * H], "u", f32r)
            bias = wtile(bias_d, [H, 8], "bias")
            whp = wtile(whp_d, [H, 2], "whp", f32r)
            bhp = wtile(bhp_d, [2, 1], "bhp")
            ident = wtile(ident_d, [H, H], "ident")

            h_sb = state.tile([H, npc], f32r, tag="h")
            c_sb = state.tile([H, npc], f32, tag="c")
            lprT = state.tile([2, npc], f32r, tag="lprT")

            # ---- prologue: transpose h0, c0, lpr into [feat, agent] ----
            for p in range(npair):
                cols = slice(p * PAIR, (p + 1) * PAIR)
                pt_h = psp.tile([128, 1024], f32, tag="ps")
                pt_c = psp.tile([128, 1024], f32, tag="ps")
                pt_l = psp.tile([128, 1024], f32, tag="ps")
                for j in range(8):
                    rows = slice(p * PAIR + j * 128, p * PAIR + (j + 1) * 128)
                    st = stage.tile([128, H], f32, tag="st_h")
                    nc.sync.dma_start(st[:], h0_d[rows, :])
                    nc.tensor.transpose(
                        pt_h[:, j * 128:(j + 1) * 128], st[:], ident[:])
                    st = stage.tile([128, H], f32, tag="st_c")
                    nc.sync.dma_start(st[:], c0_d[rows, :])
                    nc.tensor.transpose(
                        pt_c[:, j * 128:(j + 1) * 128], st[:], ident[:])
                    st = stage.tile([128, 2], f32, tag="st_l")
                    nc.sync.dma_start(st[:], lpr_d[rows, :])
                    nc.tensor.transpose(
                        pt_l[0:2, j * 128:(j + 1) * 128], st[:], ident[:])
                nc.vector.tensor_copy(h_sb[:, cols], pt_h[:])
                nc.vector.tensor_copy(c_sb[:, cols], pt_c[:])
                nc.vector.tensor_copy(lprT[:, cols], pt_l[0:2, :])

            # ---- main recurrence ----
            GATES = range(4)  # i, f, o, g
            for t in range(T):
                first = t == 0
                W = whh if first else wg
                bcol = 4 if first else 0
                xblk = outp.tile([nblk, 64], f32, tag="xblk")
                yblk = outp.tile([nblk, 64], f32, tag="yblk")
                for p in range(npair):
                    cols = slice(p * PAIR, (p + 1) * PAIR)
                    h_pr = h_sb[:, cols]
                    c_pr = c_sb[:, cols]
                    gt = [psp.tile([128, 1024], f32, tag="ps", name=f"gt{g}")
                          for g in GATES]
                    for g in GATES:
                        wsl = slice(g * H, (g + 1) * H)
                        for half in range(2):
                            hs = slice((p * 2 + half) * CH,
                                       (p * 2 + half + 1) * CH)
                            osl = slice(half * CH, (half + 1) * CH)
                            if first:
                                nc.tensor.matmul(
                                    gt[g][:, osl], u[:, wsl], lprT[:, hs],
                                    start=True, stop=False)
                            nc.tensor.matmul(
                                gt[g][:, osl], W[:, wsl], h_sb[:, hs],
                                start=not first, stop=True)

                    # activations, per-gate bias fused
                    si = sigp.tile([128, PAIR], f32, tag="si")
                    sf = sigp.tile([128, PAIR], f32, tag="sf")
                    so = sigp.tile([128, PAIR], f32, tag="so")
                    tg = sigp.tile([128, PAIR], f32, tag="tg")
                    nc.scalar.activation(si[:], gt[0][:], Act.Sigmoid,
                                         bias=bias[:, bcol:bcol + 1])
                    nc.scalar.activation(sf[:], gt[1][:], Act.Sigmoid,
                                         bias=bias[:, bcol + 1:bcol + 2])
                    nc.scalar.activation(so[:], gt[2][:], Act.Sigmoid,
                                         bias=bias[:, bcol + 2:bcol + 3])
                    nc.scalar.activation(tg[:], gt[3][:], Act.Tanh,
                                         bias=bias[:, bcol + 3:bcol + 4])

                    # cell update: c = sf*c + si*tg ; h = so*tanh(c)
                    m1 = tmpp.tile([128, PAIR], f32, tag="m1")
                    nc.vector.tensor_mul(m1[:], sf[:], c_pr)
                    m2 = tmpp.tile([128, PAIR], f32, tag="m2")
                    nc.vector.tensor_mul(m2[:], si[:], tg[:])
                    nc.gpsimd.tensor_add(c_pr, m1[:], m2[:])
                    tcl = sigp.tile([128, PAIR], f32, tag="tc")
                    nc.scalar.activation(tcl[:], c_pr, Act.Tanh)
                    # h-mul split between DVE and GPSIMD
                    nc.vector.tensor_mul(
                        h_pr[:, 0:CH], so[:, 0:CH], tcl[:, 0:CH])
                    nc.gpsimd.tensor_mul(
                        h_pr[:, CH:PAIR], so[:, CH:PAIR], tcl[:, CH:PAIR])

                    # rel = w_hp @ h + b_hp  -> [2, PAIR] psum
                    rp = psp.tile([128, 1024], f32, tag="ps")
                    for half in range(2):
                        hs = slice((p * 2 + half) * CH,
                                   (p * 2 + half + 1) * CH)
                        osl = slice(half * CH, (half + 1) * CH)
                        nc.tensor.matmul(
                            rp[0:2, osl], whp[:], h_sb[:, hs],
                            start=True, stop=True)
                    ex = tmpp.tile([2, PAIR], f32, tag="ex")
                    nc.vector.tensor_scalar_add(ex[:], rp[0:2, :], bhp[:, 0:1])
                    # re-block: agent a -> partition a//64; pair p covers
                    # partitions [16p, 16p+16)
                    prt = slice(16 * p, 16 * (p + 1))
                    nc.sync.dma_start(xblk[prt, :], ex[0:1, :])
                    nc.sync.dma_start(yblk[prt, :], ex[1:2, :])

                # interleave x/y pairs within each partition and write out:
                # out[t, 64p + a, k] <- relpk[p, 2a + k]
                relpk = outp.tile([nblk, 128], f32, tag="relpk")
                rv = relpk[:].rearrange("q (a k) -> q a k", k=2)
                nc.vector.tensor_copy(rv[:, :, 0], xblk[:])
                nc.vector.tensor_copy(rv[:, :, 1], yblk[:])
                nc.sync.dma_start(
                    out_d[t].rearrange("(q a) k -> q (a k)", a=64), relpk[:])

    nc.compile()
    return nc


def _fold_weights(w_ih, w_hh, b_ih, b_hh, w_se, b_se, w_hp, b_hp):
    """Host-side constant folding. Gate order [i, f, o, g] (torch order in
    the 4H rows is i, f, g, o)."""
    perm = np.concatenate([
        np.arange(0, H), np.arange(H, 2 * H),
        np.arange(3 * H, 4 * H), np.arange(2 * H, 3 * H),
    ])
    W_eff = w_hh + w_ih @ w_se @ w_hp                      # [4H, H]
    b_eff = (b_hp @ w_se.T + b_se) @ w_ih.T + b_ih + b_hh  # [4H]
    U = w_ih @ w_se                                        # [4H, 2]
    b1 = b_se @ w_ih.T + b_ih + b_hh                       # [4H]

    Wp, bp = W_eff[perm], b_eff[perm]
    Whhp, Up, b1p = w_hh[perm], U[perm], b1[perm]
    f = np.float32
    bias = np.stack([bp[0:H], bp[H:2*H], bp[2*H:3*H], bp[3*H:4*H],
                     b1p[0:H], b1p[H:2*H], b1p[2*H:3*H], b1p[3*H:4*H]],
                    axis=1)  # [H, 8]
    return {
        "wg": np.ascontiguousarray(Wp.T, f),
        "whh": np.ascontiguousarray(Whhp.T, f),
        "u": np.ascontiguousarray(Up.T, f),
        "bias": np.ascontiguousarray(bias, f),
        "whp": np.ascontiguousarray(w_hp.T, f),
        "bhp": np.ascontiguousarray(np.asarray(b_hp)[:, None], f),
        "ident": np.eye(H, dtype=f),
    }


def kernel(last_pos, last_pos_rel, h0, c0,
           w_ih, w_hh, b_ih, b_hh, w_se, b_se, w_hp, b_hp):
    last_pos_rel = np.ascontiguousarray(np.asarray(last_pos_rel), np.float32)
    h0 = np.ascontiguousarray(np.asarray(h0), np.float32)
    c0 = np.ascontiguousarray(np.asarray(c0), np.float32)
    consts = _fold_weights(
        np.asarray(w_ih, np.float32), np.asarray(w_hh, np.float32),
        np.asarray(b_ih, np.float32), np.asarray(b_hh, np.float32),
        np.asarray(w_se, np.float32), np.asarray(b_se, np.float32),
        np.asarray(w_hp, np.float32), np.asarray(b_hp, np.float32),
    )

    npeds = h0.shape[0]
    npc = npeds // NCORES
    if "nc" not in _CACHE or _CACHE.get("npc") != npc:
        _CACHE["nc"] = _build_program(npc)
        _CACHE["npc"] = npc
    nc = _CACHE["nc"]

    in_maps = []
    for ci in range(NCORES):
        rows = slice(ci * npc, (ci + 1) * npc)
        m = {"h0": h0[rows], "c0": c0[rows], "lpr": last_pos_rel[rows]}
        m.update(consts)
        in_maps.append(m)

    from concourse.bass_utils import run_bass_kernel_spmd
    import os

    res = run_bass_kernel_spmd(
        nc, in_maps, list(range(NCORES)),
        tmpdir=os.environ.get("KERNEL_TRACE_DIR"),
    )
    _CACHE["exec_time_ns"] = res.exec_time_ns
    _CACHE["results"] = res
    outs = [np.asarray(res.results[i]["out"]) for i in range(NCORES)]
    return np.concatenate(outs, axis=1)


# revision 10
# speedup vs baseline: 3.9793x; 1.6199x over previous
"""Trainium2 Bass kernel for the nn_Decoder LSTM-decoder problem.

Reference computation (per agent, 12 steps):
    gates = dec_in @ w_ih.T + h @ w_hh.T + (b_ih + b_hh)
    i, f, g, o = split(gates); c = sig(f)*c + sig(i)*tanh(g); h = sig(o)*tanh(c)
    rel = h @ w_hp.T + b_hp; dec_in = rel @ w_se.T + b_se
Output: rel per step, [12, N, 2].

Key algebraic fusion: dec_in_t is a linear function of h_t, so for steps >= 2
    gates_t = h_{t-1} @ W_eff.T + b_eff,  W_eff = w_hh + w_ih @ w_se @ w_hp
and step 1 uses w_hh plus U = w_ih @ w_se applied to last_pos_rel.
last_pos is dead (never affects the output).

Distribution: pure data parallel over the agent axis, 8192 agents per core
on 8 NeuronCores; weights replicated.

On-chip layout: [feature partitions, agent free]. Agents are processed in
1024-agent pairs (one [128, 1024] PSUM tile per gate) so each ACT
instruction covers 1024 elements per lane with the per-gate per-partition
bias fused. PE does float32r matmuls; DVE+GPSIMD split the cell-update
elementwise work. PSUM: gate tiles rotate through 3 slots (6 banks) and the
tiny rel matmul output has its own slot, so gate allocation never waits on
a prior pair's chain tail. The per-step rel output is re-blocked via
SBUF->SBUF DMA and pair-interleaved on DVE so the final DRAM write has
512-byte contiguous runs spread across all 16 DMA ports.
"""

import sys

if "/opt/trn_rl_repo" not in sys.path:
    sys.path.insert(0, "/opt/trn_rl_repo")

import numpy as np

T = 12          # steps
H = 128         # hidden dim
NCORES = 8
NPC = 8192      # agents per core
CH = 512        # agents per chunk (one PSUM bank at fp32)
PAIR = 2 * CH   # agents per gate-tile

_CACHE = {}


def _build_program(npc):
    import concourse.bass as bass
    import concourse.tile as tile
    from concourse import bacc, mybir

    dt = mybir.dt
    f32 = dt.float32
    f32r = dt.float32r
    Act = mybir.ActivationFunctionType

    npair = npc // PAIR
    assert npc % PAIR == 0
    nblk = npc // 64   # output partition blocks (64 agents each)

    nc = bacc.Bacc(
        "TRN2",
        target_bir_lowering=False,
        debug=False,
        num_devices=NCORES,
    )

    def din(name, shape, dt_=None):
        return nc.dram_tensor(
            name, list(shape), dt_ or f32, kind="ExternalInput"
        ).ap()

    h0_d = din("h0", [npc, H])
    c0_d = din("c0", [npc, H])
    lpr_d = din("lpr", [npc, 2])
    # lhsT layouts, K on partitions. Gate order [i, f, o, g].
    wg_d = din("wg", [H, 4 * H], f32r)    # W_eff.T columns gate-ordered
    whh_d = din("whh", [H, 4 * H], f32r)  # w_hh.T (step 1)
    u_d = din("u", [2, 4 * H], f32r)      # (w_ih @ w_se).T (step 1)
    bias_d = din("bias", [H, 8])          # ACT bias: [b_eff | b1] x [i,f,o,g]
    whp_d = din("whp", [H, 2], f32r)      # w_hp.T
    bhp_d = din("bhp", [2, 1])
    ident_d = din("ident", [H, H])
    out_d = nc.dram_tensor("out", [T, npc, 2], f32, kind="ExternalOutput").ap()

    with tile.TileContext(nc) as tc:
        with (
            tc.tile_pool(name="wpool", bufs=1) as wp,
            tc.tile_pool(name="state", bufs=1) as state,
            tc.tile_pool(name="stage", bufs=4) as stage,
            tc.tile_pool(name="sig", bufs=3) as sigp,
            tc.tile_pool(name="tmp", bufs=3) as tmpp,
            tc.tile_pool(name="outp", bufs=2) as outp,
            tc.tile_pool(name="ps", bufs=3, space="PSUM") as psp,
            tc.tile_pool(name="psr", bufs=1, space="PSUM") as psr,
        ):
            def wtile(ap, shape, tag, dt_=None):
                t_ = wp.tile(list(shape), dt_ or f32, tag=tag)
                nc.sync.dma_start(t_[:], ap)
                return t_

            wg = wtile(wg_d, [H, 4 * H], "wg", f32r)
            whh = wtile(whh_d, [H, 4 * H], "whh", f32r)
            u = wtile(u_d, [2, 4 * H], "u", f32r)
            bias = wtile(bias_d, [H, 8], "bias")
            whp = wtile(whp_d, [H, 2], "whp", f32r)
            bhp = wtile(bhp_d, [2, 1], "bhp")
            ident = wtile(ident_d, [H, H], "ident")

            h_sb = state.tile([H, npc], f32r, tag="h")
            c_sb = state.tile([H, npc], f32, tag="c")

            def step_pair(t, p, xblk, yblk, lpr_t):
                """Emit one (step, agent-pair) unit of the recurrence."""
                first = t == 0
                W = whh if first else wg
                bcol = 4 if first else 0
                cols = slice(p * PAIR, (p + 1) * PAIR)
                h_pr = h_sb[:, cols]
                c_pr = c_sb[:, cols]
                gt = [psp.tile([128, 1024], f32, tag="ps", name=f"gt{g}")
                      for g in range(4)]
                for g in range(4):
                    wsl = slice(g * H, (g + 1) * H)
                    for half in range(2):
                        hs = slice((p * 2 + half) * CH,
                                   (p * 2 + half + 1) * CH)
                        osl = slice(half * CH, (half + 1) * CH)
                        if first:
                            nc.tensor.matmul(
                                gt[g][:, osl], u[:, wsl],
                                lpr_t[:, osl],
                                start=True, stop=False)
                        nc.tensor.matmul(
                            gt[g][:, osl], W[:, wsl], h_sb[:, hs],
                            start=not first, stop=True)

                # activations, per-gate bias fused
                si = sigp.tile([128, PAIR], f32, tag="si")
                sf = sigp.tile([128, PAIR], f32, tag="sf")
                so = sigp.tile([128, PAIR], f32, tag="so")
                tg = sigp.tile([128, PAIR], f32, tag="tg")
                nc.scalar.activation(si[:], gt[0][:], Act.Sigmoid,
                                     bias=bias[:, bcol:bcol + 1])
                nc.scalar.activation(sf[:], gt[1][:], Act.Sigmoid,
                                     bias=bias[:, bcol + 1:bcol + 2])
                nc.scalar.activation(so[:], gt[2][:], Act.Sigmoid,
                                     bias=bias[:, bcol + 2:bcol + 3])
                nc.scalar.activation(tg[:], gt[3][:], Act.Tanh,
                                     bias=bias[:, bcol + 3:bcol + 4])

                # cell update: c = sf*c + si*tg ; h = so*tanh(c)
                m1 = tmpp.tile([128, PAIR], f32, tag="m1")
                nc.vector.tensor_mul(m1[:], sf[:], c_pr)
                m2 = tmpp.tile([128, PAIR], f32, tag="m2")
                nc.vector.tensor_mul(m2[:], si[:], tg[:])
                nc.gpsimd.tensor_add(c_pr, m1[:], m2[:])
                tcl = sigp.tile([128, PAIR], f32, tag="tc")
                nc.scalar.activation(tcl[:], c_pr, Act.Tanh)
                nc.vector.tensor_mul(
                    h_pr[:, 0:CH], so[:, 0:CH], tcl[:, 0:CH])
                nc.gpsimd.tensor_mul(
                    h_pr[:, CH:PAIR], so[:, CH:PAIR], tcl[:, CH:PAIR])

                # rel = w_hp @ h + b_hp  -> [2, PAIR] psum
                rp = psr.tile([2, 1024], f32, tag="rel")
                for half in range(2):
                    hs = slice((p * 2 + half) * CH,
                               (p * 2 + half + 1) * CH)
                    osl = slice(half * CH, (half + 1) * CH)
                    nc.tensor.matmul(
                        rp[0:2, osl], whp[:], h_sb[:, hs],
                        start=True, stop=True)
                ex = tmpp.tile([2, PAIR], f32, tag="ex")
                nc.vector.tensor_scalar_add(ex[:], rp[0:2, :], bhp[:, 0:1])
                # re-block: agent a -> partition a//64; pair p covers
                # partitions [16p, 16p+16)
                prt = slice(16 * p, 16 * (p + 1))
                nc.sync.dma_start(xblk[prt, :], ex[0:1, :])
                nc.sync.dma_start(yblk[prt, :], ex[1:2, :])

            def flush_step(t, xblk, yblk):
                # interleave x/y pairs per partition and write out:
                # out[t, 64q + a, k] <- relpk[q, 2a + k]
                relpk = outp.tile([nblk, 128], f32, tag="relpk")
                rv = relpk[:].rearrange("q (a k) -> q a k", k=2)
                nc.vector.tensor_copy(rv[:, :, 0], xblk[:])
                nc.vector.tensor_copy(rv[:, :, 1], yblk[:])
                nc.sync.dma_start(
                    out_d[t].rearrange("(q a) k -> q (a k)", a=64), relpk[:])

            # ---- prologue + step 0, per pair ----
            xblk = outp.tile([nblk, 64], f32, tag="xblk")
            yblk = outp.tile([nblk, 64], f32, tag="yblk")
            for p in range(npair):
                cols = slice(p * PAIR, (p + 1) * PAIR)
                pt_h = psp.tile([128, 1024], f32, tag="ps")
                pt_c = psp.tile([128, 1024], f32, tag="ps")
                pt_l = psp.tile([128, 1024], f32, tag="ps")
                for j in range(8):
                    rows = slice(p * PAIR + j * 128, p * PAIR + (j + 1) * 128)
                    st = stage.tile([128, H], f32, tag="st_h")
                    nc.sync.dma_start(st[:], h0_d[rows, :])
                    nc.tensor.transpose(
                        pt_h[:, j * 128:(j + 1) * 128], st[:], ident[:])
                    st = stage.tile([128, H], f32, tag="st_c")
                    nc.sync.dma_start(st[:], c0_d[rows, :])
                    nc.tensor.transpose(
                        pt_c[:, j * 128:(j + 1) * 128], st[:], ident[:])
                    st = stage.tile([128, 2], f32, tag="st_l")
                    nc.sync.dma_start(st[:], lpr_d[rows, :])
                    nc.tensor.transpose(
                        pt_l[0:2, j * 128:(j + 1) * 128], st[:], ident[:])
                nc.vector.tensor_copy(h_sb[:, cols], pt_h[:])
                nc.vector.tensor_copy(c_sb[:, cols], pt_c[:])
                lpr_t = tmpp.tile([2, PAIR], f32r, tag="lprp")
                nc.vector.tensor_copy(lpr_t[:], pt_l[0:2, :])
                step_pair(0, p, xblk, yblk, lpr_t)
            flush_step(0, xblk, yblk)

            # ---- steps 1..T-1 ----
            for t in range(1, T):
                xblk = outp.tile([nblk, 64], f32, tag="xblk")
                yblk = outp.tile([nblk, 64], f32, tag="yblk")
                for p in range(npair):
                    step_pair(t, p, xblk, yblk, None)
                flush_step(t, xblk, yblk)

    nc.compile()
    return nc


def _fold_weights(w_ih, w_hh, b_ih, b_hh, w_se, b_se, w_hp, b_hp):
    """Host-side constant folding. Gate order [i, f, o, g] (torch order in
    the 4H rows is i, f, g, o)."""
    perm = np.concatenate([
        np.arange(0, H), np.arange(H, 2 * H),
        np.arange(3 * H, 4 * H), np.arange(2 * H, 3 * H),
    ])
    W_eff = w_hh + w_ih @ w_se @ w_hp                      # [4H, H]
    b_eff = (b_hp @ w_se.T + b_se) @ w_ih.T + b_ih + b_hh  # [4H]
    U = w_ih @ w_se                                        # [4H, 2]
    b1 = b_se @ w_ih.T + b_ih + b_hh                       # [4H]

    Wp, bp = W_eff[perm], b_eff[perm]
    Whhp, Up, b1p = w_hh[perm], U[perm], b1[perm]
    f = np.float32
    bias = np.stack([bp[0:H], bp[H:2*H], bp[2*H:3*H], bp[3*H:4*H],
                     b1p[0:H], b1p[H:2*H], b1p[2*H:3*H], b1p[3*H:4*H]],
                    axis=1)  # [H, 8]
    return {
        "wg": np.ascontiguousarray(Wp.T, f),
        "whh": np.ascontiguousarray(Whhp.T, f),
        "u": np.ascontiguousarray(Up.T, f),
        "bias": np.ascontiguousarray(bias, f),
        "whp": np.ascontiguousarray(w_hp.T, f),
        "bhp": np.ascontiguousarray(np.asarray(b_hp)[:, None], f),
        "ident": np.eye(H, dtype=f),
    }


def kernel(last_pos, last_pos_rel, h0, c0,
           w_ih, w_hh, b_ih, b_hh, w_se, b_se, w_hp, b_hp):
    last_pos_rel = np.ascontiguousarray(np.asarray(last_pos_rel), np.float32)
    h0 = np.ascontiguousarray(np.asarray(h0), np.float32)
    c0 = np.ascontiguousarray(np.asarray(c0), np.float32)
    consts = _fold_weights(
        np.asarray(w_ih, np.float32), np.asarray(w_hh, np.float32),
        np.asarray(b_ih, np.float32), np.asarray(b_hh, np.float32),
        np.asarray(w_se, np.float32), np.asarray(b_se, np.float32),
        np.asarray(w_hp, np.float32), np.asarray(b_hp, np.float32),
    )

    npeds = h0.shape[0]
    npc = npeds // NCORES
    if "nc" not in _CACHE or _CACHE.get("npc") != npc:
        _CACHE["nc"] = _build_program(npc)
        _CACHE["npc"] = npc
    nc = _CACHE["nc"]

    in_maps = []
    for ci in range(NCORES):
        rows = slice(ci * npc, (ci + 1) * npc)
        m = {"h0": h0[rows], "c0": c0[rows], "lpr": last_pos_rel[rows]}
        m.update(consts)
        in_maps.append(m)

    from concourse.bass_utils import run_bass_kernel_spmd
    import os

    res = run_bass_kernel_spmd(
        nc, in_maps, list(range(NCORES)),
        tmpdir=os.environ.get("KERNEL_TRACE_DIR"),
    )
    _CACHE["exec_time_ns"] = res.exec_time_ns
    _CACHE["results"] = res
    outs = [np.asarray(res.results[i]["out"]) for i in range(NCORES)]
    return np.concatenate(outs, axis=1)
